# revision 27
# baseline (speedup 1.0000x reference)
"""Trainium2 Bass kernel for fused MHA block (QKV -> masked softmax attention
-> out-proj -> residual -> LayerNorm), sharded over 8 NeuronCores.

Zero-communication sharding: core c handles batch b=c//4 and query rows
[512*r, 512*(r+1)) with r=c%4. Each core recomputes K,V for the FULL 2048
rows of its batch from the (replicated) input x -- no collectives at all --
and computes Q only for its own 512 rows. Attention runs transposed
([k, q] scores, so no on-chip transposes), mask is applied as a post-exp
0/1 multiply on the vector engine, and the softmax denominator comes from
an appended ones-column in V. Everything is bf16 on the PE with fp32 PSUM
accumulation; LayerNorm is fp32.

Self-contained: hardcodes all shapes; only needs numpy/ml_dtypes/concourse.
"""

import numpy as np
import ml_dtypes

from concourse import bacc, bass_utils, mybir, tile
import concourse.bass as bass

B, S, D = 2, 2048, 1024
H, DH = 16, 64
SL = 512  # per-core query-row shard
NCORES = 8
R = 4
PAIRS = [[0, 1], [2, 3], [4, 5], [6, 7]]

f32 = mybir.dt.float32
bf16 = mybir.dt.bfloat16
AF = mybir.ActivationFunctionType
ALU = mybir.AluOpType


def _build():
    nc = bacc.Bacc("TRN2", target_bir_lowering=False, debug=False,
                   num_devices=NCORES)

    xTh = nc.dram_tensor("xTh", [D, 2 * SL], bf16, kind="ExternalInput")
    xTq = nc.dram_tensor("xTq", [D, SL], bf16, kind="ExternalInput")
    wq = nc.dram_tensor("wq", [D, D], bf16, kind="ExternalInput")
    wk = nc.dram_tensor("wk", [D, D], bf16, kind="ExternalInput")
    wv = nc.dram_tensor("wv", [D, D], bf16, kind="ExternalInput")
    bq = nc.dram_tensor("bq", [128, 8], f32, kind="ExternalInput")
    bk = nc.dram_tensor("bk", [128, 8], f32, kind="ExternalInput")
    bv = nc.dram_tensor("bv", [1, D], f32, kind="ExternalInput")
    wout = nc.dram_tensor("wout", [D, D], bf16, kind="ExternalInput")
    maskm = nc.dram_tensor("maskm", [S, SL], bf16, kind="ExternalInput")
    xres = nc.dram_tensor("xres", [SL, D], f32, kind="ExternalInput")
    lng = nc.dram_tensor("lng", [1, D], f32, kind="ExternalInput")
    lnb = nc.dram_tensor("lnb", [1, D], f32, kind="ExternalInput")
    out = nc.dram_tensor("out", [SL, D], f32, kind="ExternalOutput")

    with tile.TileContext(nc) as tc:
        _body(tc, nc, xTh, xTq, wq, wk, wv, bq, bk, bv, wout, maskm, xres,
              lng, lnb, out)
    nc.compile()
    return nc


def _body(tc, nc, xTh, xTq, wq, wk, wv, bq, bk, bv, wout, maskm, xres,
          lng, lnb, out):
    VW = 65  # per-head V stripe: 64 v dims + 1 ones column (denominator)
    with (
        tc.tile_pool(name="singles", bufs=1) as singles,
        tc.tile_pool(name="dpool", bufs=1, space="DRAM") as dpool,
    ):
        dn_dram = dpool.tile([8, 2, SL], f32)
        CH2 = SL * D  # elements per K (or V) subchunk piece
        kv_loc = [dpool.tile([2, CH2], bf16, name=f"kvloc{s}")
                  for s in range(2)]
        kv_ag = [dpool.tile([2, 2, CH2], bf16, name=f"kvag{s}")
                 for s in range(2)]
        # ---- long-lived tiles ----
        bqs = singles.tile([128, 8], f32)
        nc.sync.dma_start(out=bqs, in_=bq.ap())
        bks = singles.tile([128, 8], f32)
        nc.sync.dma_start(out=bks, in_=bk.ap())

        kT_sb = singles.tile([128, 8, S], bf16)     # K^T full batch
        v_sb = singles.tile([128, 16, H * VW], bf16)  # V full, ones-striped
        qT_sb = singles.tile([128, 8, SL], bf16)    # Q^T own rows
        attnT_sb = singles.tile([128, 8, SL], bf16)
        maskm_sb = singles.tile([128, 16, SL], bf16)
        bvb = singles.tile([128, D], f32)
        lngb = singles.tile([128, D], f32)
        lnbb = singles.tile([128, D], f32)
        epss = singles.tile([128, 1], f32)
        nc.vector.memset(epss, 1e-5)
        # ones columns for the softmax-denominator trick (V evac writes
        # only the 64-wide v stripes, leaving col 64 of each stripe at 1)
        nc.vector.memset(v_sb, 1.0)

        # ---- phase 1: Q (own rows), then K,V for the full batch ----
        with (
            tc.tile_pool(name="p1x", bufs=1) as p1x,
            tc.tile_pool(name="p1w", bufs=2) as p1w,
            tc.tile_pool(name="p1c", bufs=2) as p1c,
            tc.tile_pool(name="p1ps", bufs=2, space="PSUM") as p1ps,
            tc.tile_pool(name="p1psv", bufs=2, space="PSUM") as p1psv,
        ):
            xTq_sb = p1x.tile([128, 8, SL], bf16)
            for kt in range(8):
                nc.sync.dma_start(
                    out=xTq_sb[:, kt, :],
                    in_=xTq.ap()[:, :].rearrange("(t p) q -> p t q", p=128)
                    [:, kt, :],
                )
            wq_sb = p1w.tile([128, 8, D], bf16, name="wq", tag="w")
            nc.sync.dma_start(
                out=wq_sb, in_=wq.ap().rearrange("(t p) f -> p t f", p=128)
            )
            xTh_sb = p1x.tile([128, 8, 2 * SL], bf16)
            for kt in range(8):
                nc.sync.dma_start(
                    out=xTh_sb[:, kt, :],
                    in_=xTh.ap().rearrange("(t p) n -> p t n", p=128)
                    [:, kt, :],
                )
            wk_sb = p1w.tile([128, 8, D], bf16, name="wk", tag="w")
            nc.sync.dma_start(
                out=wk_sb, in_=wk.ap().rearrange("(t p) f -> p t f", p=128)
            )
            wv_sb = p1w.tile([128, 8, D], bf16, name="wv", tag="w")
            nc.sync.dma_start(
                out=wv_sb, in_=wv.ap().rearrange("(t p) f -> p t f", p=128)
            )
            nc.sync.dma_start(out=bvb, in_=bv.ap().to_broadcast([128, D]))
            nc.sync.dma_start(
                out=maskm_sb,
                in_=maskm.ap().rearrange("(t p) q -> p t q", p=128),
            )
            nc.sync.dma_start(out=lngb, in_=lng.ap().to_broadcast([128, D]))
            nc.sync.dma_start(out=lnbb, in_=lnb.ap().to_broadcast([128, D]))

            # K^T,V for own 1024-row half, in two 512-row subchunks,
            # each AllGathered (bf16) across the 2-core pair
            for s in range(2):
                ss = slice(s * SL, (s + 1) * SL)
                kch = p1c.tile([128, 8, SL], bf16, name=f"kch{s}", tag="kch")
                for pt in range(8):
                    psk = p1ps.tile([128, SL], f32, name=f"psk{s}_{pt}",
                                    tag="ps")
                    for kt in range(8):
                        nc.tensor.matmul(
                            psk,
                            wk_sb[:, kt, pt * 128:(pt + 1) * 128],
                            xTh_sb[:, kt, ss],
                            start=(kt == 0), stop=(kt == 7),
                        )
                    nc.scalar.activation(
                        out=kch[:, pt, :], in_=psk, func=AF.Identity,
                        bias=bks[:, pt:pt + 1], scale=1.0,
                    )
                nc.sync.dma_start(
                    out=kv_loc[s][0, :].rearrange("(t p f) -> p t f",
                                                  p=128, f=SL),
                    in_=kch,
                )
                for st2 in range(4):
                    col = s * SL + st2 * 128
                    psv = p1psv.tile([128, D], f32, name=f"psv{s}_{st2}",
                                    tag="psv")
                    for q2 in range(2):
                        for kt in range(8):
                            nc.tensor.matmul(
                                psv[:, q2 * SL:(q2 + 1) * SL],
                                xTh_sb[:, kt, col:col + 128],
                                wv_sb[:, kt, q2 * SL:(q2 + 1) * SL],
                                start=(kt == 0), stop=(kt == 7),
                            )
                    vch = p1c.tile([128, D], bf16, name=f"vch{s}_{st2}",
                                   tag="vch")
                    nc.vector.tensor_add(out=vch, in0=psv, in1=bvb)
                    nc.sync.dma_start(
                        out=kv_loc[s][1, st2 * 128 * D:(st2 + 1) * 128 * D]
                        .rearrange("(p f) -> p f", p=128),
                        in_=vch,
                    )
                nc.gpsimd.collective_compute(
                    "AllGather", ALU.bypass, replica_groups=PAIRS,
                    ins=[kv_loc[s].opt()], outs=[kv_ag[s].opt()],
                )

            # Q^T: out [1024 feat rows, 512 q]
            for pt in range(8):
                psq = p1ps.tile([128, SL], f32, name=f"psq{pt}", tag="ps")
                for kt in range(8):
                    nc.tensor.matmul(
                        psq,
                        wq_sb[:, kt, pt * 128:(pt + 1) * 128],
                        xTq_sb[:, kt, :],
                        start=(kt == 0), stop=(kt == 7),
                    )
                nc.scalar.activation(
                    out=qT_sb[:, pt, :], in_=psq, func=AF.Identity,
                    bias=bqs[:, pt:pt + 1], scale=1.0,
                )

            # unpack: gathered position 1024*s + 512*j + p holds global
            # k row 1024*j + 512*s + p (mask rows permuted to match)
            for s in range(2):
                for j in range(2):
                    pos = 2 * s + j  # 512-row block index in gathered order
                    nc.sync.dma_start(
                        out=kT_sb[:, :, pos * SL:(pos + 1) * SL],
                        in_=kv_ag[s][j, 0, :]
                        .rearrange("(t p f) -> p t f", p=128, f=SL),
                    )
                    for st2 in range(4):
                        g = pos * 4 + st2
                        nc.sync.dma_start(
                            out=v_sb[:, g, :]
                            .rearrange("p (h c) -> p h c", c=VW)[:, :, 0:64],
                            in_=kv_ag[s][j, 1,
                                         st2 * 128 * D:(st2 + 1) * 128 * D]
                            .rearrange("(p h c) -> p h c", p=128, c=64),
                        )

        # ---- phase 2: attention, head pairs, scores transposed [k, q] ----
        with (
            tc.tile_pool(name="att_pr", bufs=4) as prp,
            tc.tile_pool(name="att_nm", bufs=3) as nmp,
            tc.tile_pool(name="att_ps", bufs=2, space="PSUM") as psp,
            tc.tile_pool(name="att_av", bufs=2, space="PSUM") as avp,
        ):
            for hg in range(8):  # head pair: heads 2*hg, 2*hg+1
                avs = avp.tile([128, 2, SL], f32, name=f"avs{hg}", tag="av")
                for kt in range(16):
                    ps = psp.tile([128, 2, SL], f32, name=f"ps{hg}_{kt}",
                                  tag="ps")
                    for i in range(2):
                        po = i * 64
                        nc.tensor.matmul(
                            ps[:, i, :],
                            kT_sb[po:po + 64, hg, kt * 128:(kt + 1) * 128],
                            qT_sb[po:po + 64, hg, :],
                            start=True, stop=True,
                        )
                    pr = prp.tile([128, 2, SL], bf16, name=f"pr{hg}_{kt}",
                                  tag="pr")
                    nc.scalar.activation(out=pr, in_=ps, func=AF.Exp,
                                         scale=0.125)
                    ms = maskm_sb[:, kt, :]
                    nc.vector.tensor_mul(
                        out=pr, in0=pr,
                        in1=bass.AP(tensor=ms.tensor, offset=ms.offset,
                                    ap=[list(ms.ap[0]), [0, 2],
                                        list(ms.ap[1])]),
                    )
                    for i in range(2):
                        h = 2 * hg + i
                        nc.tensor.matmul(
                            avs[0:VW, i, :],
                            v_sb[:, kt, h * VW:(h + 1) * VW],
                            pr[:, i, :],
                            start=(kt == 0), stop=(kt == 15),
                        )
                # normalize: denom (row 64) -> DRAM -> broadcast raw denoms
                # to 64 partitions -> approx reciprocal -> scale
                rc = nmp.tile([VW, 2, SL], f32, name=f"rc{hg}", tag="rc")
                nc.vector.tensor_copy(rc[64:VW, :, :], avs[64:VW, :, :])
                nc.sync.dma_start(out=dn_dram[hg, :, :], in_=rc[64:VW, :, :])
                rb = nmp.tile([64, 2, SL], f32, name=f"rb{hg}", tag="rb")
                dsrc = dn_dram[hg, :, :]
                nc.sync.dma_start(
                    out=rb,
                    in_=bass.AP(tensor=dsrc.tensor, offset=dsrc.offset,
                                ap=[[0, 64]] + [list(p) for p in dsrc.ap]),
                )
                nc.vector.reciprocal_approx_fast(out=rb, in_=rb)
                atn = nmp.tile([64, 2, SL], bf16, name=f"atn{hg}", tag="atn")
                nc.vector.tensor_mul(out=atn, in0=avs[0:64, :, :], in1=rb)
                for i in range(2):
                    nc.sync.dma_start(
                        out=attnT_sb[64 * i:64 * i + 64, hg, :],
                        in_=atn[:, i, :],
                    )

        # ---- phase 3: out-projection + residual + LayerNorm ----
        with (
            tc.tile_pool(name="p3", bufs=1) as p3,
            tc.tile_pool(name="op_ps", bufs=8, space="PSUM") as opps,
            tc.tile_pool(name="ln", bufs=4) as lnp,
        ):
            wout_sb = p3.tile([128, 8, D], bf16)
            nc.sync.dma_start(
                out=wout_sb, in_=wout.ap().rearrange("(t p) f -> p t f", p=128)
            )
            xres_sb = p3.tile([128, 4, D], f32)
            nc.sync.dma_start(
                out=xres_sb, in_=xres.ap().rearrange("(t p) d -> p t d", p=128)
            )
            y_sb = p3.tile([128, 4, D], f32)
            for qt in range(4):
                yps = [opps.tile([128, SL], f32, name=f"yps{qt}_{nch}",
                                 tag="yps") for nch in range(2)]
                for kt in range(8):
                    for nch in range(2):
                        nc.tensor.matmul(
                            yps[nch],
                            attnT_sb[:, kt, qt * 128:(qt + 1) * 128],
                            wout_sb[:, kt, nch * SL:(nch + 1) * SL],
                            start=(kt == 0), stop=(kt == 7),
                        )
                for nch in range(2):
                    nc.vector.tensor_add(
                        out=y_sb[:, qt, nch * SL:(nch + 1) * SL],
                        in0=yps[nch],
                        in1=xres_sb[:, qt, nch * SL:(nch + 1) * SL],
                    )
                stats = lnp.tile([128, 2, 6], f32, name=f"st{qt}", tag="st")
                for i in range(2):
                    nc.vector.bn_stats(
                        out=stats[:, i, :],
                        in_=y_sb[:, qt, i * SL:(i + 1) * SL],
                    )
                mv = lnp.tile([128, 2], f32, name=f"mv{qt}", tag="mv")
                nc.vector.bn_aggr(out=mv, in_=stats)
                nc.scalar.activation(
                    out=mv[:, 1:2], in_=mv[:, 1:2], func=AF.Sqrt,
                    bias=epss, scale=1.0,
                )
                nc.vector.reciprocal(out=mv[:, 1:2], in_=mv[:, 1:2])
                yt = lnp.tile([128, D], f32, name=f"yt{qt}", tag="yt")
                nc.vector.tensor_scalar(
                    out=yt, in0=y_sb[:, qt, :], scalar1=mv[:, 0:1],
                    scalar2=mv[:, 1:2], op0=ALU.subtract, op1=ALU.mult,
                )
                nc.vector.tensor_mul(out=yt, in0=yt, in1=lngb)
                nc.vector.tensor_add(out=yt, in0=yt, in1=lnbb)
                nc.sync.dma_start(
                    out=out.ap()[qt * 128:(qt + 1) * 128, :], in_=yt
                )


_NC_CACHE = None


def kernel(**inputs) -> np.ndarray:
    global _NC_CACHE
    x = np.asarray(inputs["x"], dtype=np.float32)
    W_attn = np.asarray(inputs["W_attn"], np.float32)
    b_attn = np.asarray(inputs["b_attn"], np.float32)
    W_out = np.asarray(inputs["W_out"], np.float32)
    b_out = np.asarray(inputs["b_out"], np.float32)
    ln_g = np.asarray(inputs["ln_g"], np.float32)
    ln_b = np.asarray(inputs["ln_b"], np.float32)
    mask = np.asarray(inputs["mask"])

    if _NC_CACHE is None:
        _NC_CACHE = _build()
    nc = _NC_CACHE

    bqa = np.ascontiguousarray(b_attn[0:D].reshape(8, 128).T)
    bka = np.ascontiguousarray(b_attn[D:2 * D].reshape(8, 128).T)
    bva = np.ascontiguousarray(b_attn[2 * D:3 * D].reshape(1, D))
    wqa = np.ascontiguousarray(W_attn[:, 0:D]).astype(ml_dtypes.bfloat16)
    wka = np.ascontiguousarray(W_attn[:, D:2 * D]).astype(ml_dtypes.bfloat16)
    wva = np.ascontiguousarray(W_attn[:, 2 * D:3 * D]).astype(
        ml_dtypes.bfloat16)
    wo = np.ascontiguousarray(W_out).astype(ml_dtypes.bfloat16)
    xTb = [np.ascontiguousarray(x[b].T).astype(ml_dtypes.bfloat16)
           for b in range(B)]

    # gathered k-order: position 1024*s + 512*j + p holds global k row
    # 1024*j + 512*s + p  (s = subchunk, j = pair rank)
    kperm = np.concatenate(
        [np.arange(SL) + 1024 * (pos % 2) + SL * (pos // 2)
         for pos in range(4)]
    )
    in_maps = []
    for c in range(NCORES):
        b, r = divmod(c, R)
        rows = slice(SL * r, SL * (r + 1))
        half = slice(1024 * (c % 2), 1024 * (c % 2) + 1024)
        xTql = np.ascontiguousarray(xTb[b][:, rows])
        xThl = np.ascontiguousarray(xTb[b][:, half])
        keep = (~mask[b, 0, rows, :]).T.astype(np.float32)  # [k, q]
        mKeep = np.ascontiguousarray(keep[kperm, :]).astype(
            ml_dtypes.bfloat16)
        xresl = np.ascontiguousarray(x[b, rows, :] + b_out[None, :])
        in_maps.append(dict(
            xTh=xThl, xTq=xTql, wq=wqa, wk=wka, wv=wva, bq=bqa, bk=bka,
            bv=bva, wout=wo, maskm=mKeep, xres=xresl,
            lng=ln_g.reshape(1, D), lnb=ln_b.reshape(1, D),
        ))

    res = bass_utils.run_bass_kernel_spmd(nc, in_maps,
                                          core_ids=list(range(NCORES)))
    kernel.last_results = res

    full = np.empty((B, S, D), np.float32)
    for c in range(NCORES):
        b, r = divmod(c, R)
        full[b, SL * r:SL * (r + 1), :] = res.results[c]["out"]
    return full


if __name__ == "__main__":
    rng = np.random.default_rng(0)
    ins = dict(
        x=rng.standard_normal((B, S, D), dtype=np.float32),
        W_attn=rng.standard_normal((D, 3 * D), dtype=np.float32) / 32,
        b_attn=np.zeros(3 * D, np.float32),
        W_out=rng.standard_normal((D, D), dtype=np.float32) / 32,
        b_out=np.zeros(D, np.float32),
        ln_g=np.ones(D, np.float32),
        ln_b=np.zeros(D, np.float32),
        mask=rng.integers(0, 5, (B, 1, S, S)) == 0,
    )
    y = kernel(**ins)
    print("ok", y.shape, y.dtype)


# revision 28
# speedup vs baseline: 1.1365x; 1.1365x over previous
"""Trainium2 Bass kernel for fused MHA block (QKV -> masked softmax attention
-> out-proj -> residual -> LayerNorm), sharded over 8 NeuronCores.

Zero-communication sharding: core c handles batch b=c//4 and query rows
[512*r, 512*(r+1)) with r=c%4. Each core recomputes K,V for the FULL 2048
rows of its batch from the (replicated) input x -- no collectives at all --
and computes Q only for its own 512 rows. Attention runs transposed
([k, q] scores, so no on-chip transposes), mask is applied as a post-exp
0/1 multiply on the vector engine, and the softmax denominator comes from
an appended ones-column in V. Everything is bf16 on the PE with fp32 PSUM
accumulation; LayerNorm is fp32.

Self-contained: hardcodes all shapes; only needs numpy/ml_dtypes/concourse.
"""

import numpy as np
import ml_dtypes

from concourse import bacc, bass_utils, mybir, tile
import concourse.bass as bass

B, S, D = 2, 2048, 1024
H, DH = 16, 64
SL = 512  # per-core query-row shard
NCORES = 8
R = 4

f32 = mybir.dt.float32
bf16 = mybir.dt.bfloat16
AF = mybir.ActivationFunctionType
ALU = mybir.AluOpType


def _build():
    nc = bacc.Bacc("TRN2", target_bir_lowering=False, debug=False,
                   num_devices=NCORES)

    xT = nc.dram_tensor("xT", [D, S], bf16, kind="ExternalInput")
    xTq = nc.dram_tensor("xTq", [D, SL], bf16, kind="ExternalInput")
    wq = nc.dram_tensor("wq", [D, D], bf16, kind="ExternalInput")
    wk = nc.dram_tensor("wk", [D, D], bf16, kind="ExternalInput")
    wv = nc.dram_tensor("wv", [D, D], bf16, kind="ExternalInput")
    bq = nc.dram_tensor("bq", [128, 8], f32, kind="ExternalInput")
    bk = nc.dram_tensor("bk", [128, 8], f32, kind="ExternalInput")
    bv = nc.dram_tensor("bv", [1, D], f32, kind="ExternalInput")
    wout = nc.dram_tensor("wout", [D, D], bf16, kind="ExternalInput")
    maskm = nc.dram_tensor("maskm", [S, SL], bf16, kind="ExternalInput")
    xres = nc.dram_tensor("xres", [SL, D], f32, kind="ExternalInput")
    lng = nc.dram_tensor("lng", [1, D], f32, kind="ExternalInput")
    lnb = nc.dram_tensor("lnb", [1, D], f32, kind="ExternalInput")
    out = nc.dram_tensor("out", [SL, D], f32, kind="ExternalOutput")

    with tile.TileContext(nc) as tc:
        _body(tc, nc, xT, xTq, wq, wk, wv, bq, bk, bv, wout, maskm, xres,
              lng, lnb, out)
    nc.compile()
    return nc


def _body(tc, nc, xT, xTq, wq, wk, wv, bq, bk, bv, wout, maskm, xres,
          lng, lnb, out):
    VW = 65  # per-head V stripe: 64 v dims + 1 ones column (denominator)
    with (
        tc.tile_pool(name="singles", bufs=1) as singles,
        tc.tile_pool(name="dpool", bufs=1, space="DRAM") as dpool,
    ):
        dn_dram = dpool.tile([8, 2, SL], f32)
        # ---- long-lived tiles ----
        bqs = singles.tile([128, 8], f32)
        nc.sync.dma_start(out=bqs, in_=bq.ap())
        bks = singles.tile([128, 8], f32)
        nc.sync.dma_start(out=bks, in_=bk.ap())

        kT_sb = singles.tile([128, 8, S], bf16)     # K^T full batch
        v_sb = singles.tile([128, 16, H * VW], bf16)  # V full, ones-striped
        qT_sb = singles.tile([128, 8, SL], bf16)    # Q^T own rows
        attnT_sb = singles.tile([128, 8, SL], bf16)
        maskm_sb = singles.tile([128, 16, SL], bf16)
        bvb = singles.tile([128, D], f32)
        lngb = singles.tile([128, D], f32)
        lnbb = singles.tile([128, D], f32)
        epss = singles.tile([128, 1], f32)
        nc.vector.memset(epss, 1e-5)
        # ones columns for the softmax-denominator trick (V evac writes
        # only the 64-wide v stripes, leaving col 64 of each stripe at 1)
        nc.vector.memset(v_sb, 1.0)

        # ---- phase 1: Q (own rows), then K,V for the full batch ----
        with (
            tc.tile_pool(name="p1x", bufs=1) as p1x,
            tc.tile_pool(name="p1w", bufs=2) as p1w,
            tc.tile_pool(name="p1ps", bufs=4, space="PSUM") as p1ps,
        ):
            xTq_sb = p1x.tile([128, 8, SL], bf16)
            for kt in range(8):
                nc.sync.dma_start(
                    out=xTq_sb[:, kt, :],
                    in_=xTq.ap()[:, :].rearrange("(t p) q -> p t q", p=128)
                    [:, kt, :],
                )
            wq_sb = p1w.tile([128, 8, D], bf16, name="wq", tag="w")
            nc.sync.dma_start(
                out=wq_sb, in_=wq.ap().rearrange("(t p) f -> p t f", p=128)
            )
            xT_sb = p1x.tile([128, 8, S], bf16)
            for kt in range(8):
                nc.sync.dma_start(
                    out=xT_sb[:, kt, :],
                    in_=xT.ap().rearrange("(t p) n -> p t n", p=128)[:, kt, :],
                )
            wk_sb = p1w.tile([128, 8, D], bf16, name="wk", tag="w")
            nc.sync.dma_start(
                out=wk_sb, in_=wk.ap().rearrange("(t p) f -> p t f", p=128)
            )
            wv_sb = p1w.tile([128, 8, D], bf16, name="wv", tag="w")
            nc.sync.dma_start(
                out=wv_sb, in_=wv.ap().rearrange("(t p) f -> p t f", p=128)
            )
            nc.sync.dma_start(out=bvb, in_=bv.ap().to_broadcast([128, D]))
            nc.sync.dma_start(
                out=maskm_sb,
                in_=maskm.ap().rearrange("(t p) q -> p t q", p=128),
            )
            nc.sync.dma_start(out=lngb, in_=lng.ap().to_broadcast([128, D]))
            nc.sync.dma_start(out=lnbb, in_=lnb.ap().to_broadcast([128, D]))

            # Q^T: out [1024 feat rows, 512 q]
            for pt in range(8):
                psq = p1ps.tile([128, D], f32, name=f"psq{pt}", tag="ps")
                for kt in range(8):
                    nc.tensor.matmul(
                        psq[:, 0:SL],
                        wq_sb[:, kt, pt * 128:(pt + 1) * 128],
                        xTq_sb[:, kt, :],
                        start=(kt == 0), stop=(kt == 7),
                    )
                nc.scalar.activation(
                    out=qT_sb[:, pt, :], in_=psq[:, 0:SL], func=AF.Identity,
                    bias=bqs[:, pt:pt + 1], scale=1.0,
                )

            # K^T: out [1024 feat rows, 2048 k]
            for pt in range(8):
                for hf in range(2):
                    psk = p1ps.tile([128, D], f32, name=f"psk{pt}_{hf}",
                                    tag="ps")
                    for kt in range(8):
                        for q2 in range(2):
                            nc.tensor.matmul(
                                psk[:, q2 * SL:(q2 + 1) * SL],
                                wk_sb[:, kt, pt * 128:(pt + 1) * 128],
                                xT_sb[:, kt,
                                      hf * D + q2 * SL:hf * D + (q2 + 1) * SL],
                                start=(kt == 0), stop=(kt == 7),
                            )
                    nc.scalar.activation(
                        out=kT_sb[:, pt, hf * D:(hf + 1) * D], in_=psk,
                        func=AF.Identity, bias=bks[:, pt:pt + 1], scale=1.0,
                    )

            # V: out [2048 tok rows, 1024 feats], striped into v_sb
            for st in range(16):
                psv = p1ps.tile([128, D], f32, name=f"psv{st}", tag="ps")
                for kt in range(8):
                    for q2 in range(2):
                        nc.tensor.matmul(
                            psv[:, q2 * SL:(q2 + 1) * SL],
                            xT_sb[:, kt, st * 128:(st + 1) * 128],
                            wv_sb[:, kt, q2 * SL:(q2 + 1) * SL],
                            start=(kt == 0), stop=(kt == 7),
                        )
                nc.vector.tensor_add(
                    out=v_sb[:, st, :]
                    .rearrange("p (h c) -> p h c", c=VW)[:, :, 0:64],
                    in0=psv.rearrange("p (h c) -> p h c", c=64),
                    in1=bvb.rearrange("p (h c) -> p h c", c=64),
                )

        # ---- phase 2: attention, head pairs, scores transposed [k, q] ----
        with (
            tc.tile_pool(name="att_pr", bufs=4) as prp,
            tc.tile_pool(name="att_nm", bufs=3) as nmp,
            tc.tile_pool(name="att_ps", bufs=2, space="PSUM") as psp,
            tc.tile_pool(name="att_av", bufs=2, space="PSUM") as avp,
        ):
            for hg in range(8):  # head pair: heads 2*hg, 2*hg+1
                avs = avp.tile([128, 2, SL], f32, name=f"avs{hg}", tag="av")
                for kt in range(16):
                    ps = psp.tile([128, 2, SL], f32, name=f"ps{hg}_{kt}",
                                  tag="ps")
                    for i in range(2):
                        po = i * 64
                        nc.tensor.matmul(
                            ps[:, i, :],
                            kT_sb[po:po + 64, hg, kt * 128:(kt + 1) * 128],
                            qT_sb[po:po + 64, hg, :],
                            start=True, stop=True,
                        )
                    pr = prp.tile([128, 2, SL], bf16, name=f"pr{hg}_{kt}",
                                  tag="pr")
                    nc.scalar.activation(out=pr, in_=ps, func=AF.Exp,
                                         scale=0.125)
                    ms = maskm_sb[:, kt, :]
                    nc.vector.tensor_mul(
                        out=pr, in0=pr,
                        in1=bass.AP(tensor=ms.tensor, offset=ms.offset,
                                    ap=[list(ms.ap[0]), [0, 2],
                                        list(ms.ap[1])]),
                    )
                    for i in range(2):
                        h = 2 * hg + i
                        nc.tensor.matmul(
                            avs[0:VW, i, :],
                            v_sb[:, kt, h * VW:(h + 1) * VW],
                            pr[:, i, :],
                            start=(kt == 0), stop=(kt == 15),
                        )
                # normalize: denom (row 64) -> DRAM -> broadcast raw denoms
                # to 64 partitions -> approx reciprocal -> scale
                rc = nmp.tile([VW, 2, SL], f32, name=f"rc{hg}", tag="rc")
                nc.vector.tensor_copy(rc[64:VW, :, :], avs[64:VW, :, :])
                nc.sync.dma_start(out=dn_dram[hg, :, :], in_=rc[64:VW, :, :])
                rb = nmp.tile([64, 2, SL], f32, name=f"rb{hg}", tag="rb")
                dsrc = dn_dram[hg, :, :]
                nc.sync.dma_start(
                    out=rb,
                    in_=bass.AP(tensor=dsrc.tensor, offset=dsrc.offset,
                                ap=[[0, 64]] + [list(p) for p in dsrc.ap]),
                )
                nc.vector.reciprocal_approx_fast(out=rb, in_=rb)
                atn = nmp.tile([64, 2, SL], bf16, name=f"atn{hg}", tag="atn")
                nc.vector.tensor_mul(out=atn, in0=avs[0:64, :, :], in1=rb)
                for i in range(2):
                    nc.sync.dma_start(
                        out=attnT_sb[64 * i:64 * i + 64, hg, :],
                        in_=atn[:, i, :],
                    )

        # ---- phase 3: out-projection + residual + LayerNorm ----
        with (
            tc.tile_pool(name="p3", bufs=1) as p3,
            tc.tile_pool(name="op_ps", bufs=8, space="PSUM") as opps,
            tc.tile_pool(name="ln", bufs=4) as lnp,
        ):
            wout_sb = p3.tile([128, 8, D], bf16)
            nc.sync.dma_start(
                out=wout_sb, in_=wout.ap().rearrange("(t p) f -> p t f", p=128)
            )
            xres_sb = p3.tile([128, 4, D], f32)
            nc.sync.dma_start(
                out=xres_sb, in_=xres.ap().rearrange("(t p) d -> p t d", p=128)
            )
            y_sb = p3.tile([128, 4, D], f32)
            for qt in range(4):
                yps = [opps.tile([128, SL], f32, name=f"yps{qt}_{nch}",
                                 tag="yps") for nch in range(2)]
                for kt in range(8):
                    for nch in range(2):
                        nc.tensor.matmul(
                            yps[nch],
                            attnT_sb[:, kt, qt * 128:(qt + 1) * 128],
                            wout_sb[:, kt, nch * SL:(nch + 1) * SL],
                            start=(kt == 0), stop=(kt == 7),
                        )
                for nch in range(2):
                    nc.vector.tensor_add(
                        out=y_sb[:, qt, nch * SL:(nch + 1) * SL],
                        in0=yps[nch],
                        in1=xres_sb[:, qt, nch * SL:(nch + 1) * SL],
                    )
                stats = lnp.tile([128, 2, 6], f32, name=f"st{qt}", tag="st")
                for i in range(2):
                    nc.vector.bn_stats(
                        out=stats[:, i, :],
                        in_=y_sb[:, qt, i * SL:(i + 1) * SL],
                    )
                mv = lnp.tile([128, 2], f32, name=f"mv{qt}", tag="mv")
                nc.vector.bn_aggr(out=mv, in_=stats)
                nc.scalar.activation(
                    out=mv[:, 1:2], in_=mv[:, 1:2], func=AF.Sqrt,
                    bias=epss, scale=1.0,
                )
                nc.vector.reciprocal(out=mv[:, 1:2], in_=mv[:, 1:2])
                yt = lnp.tile([128, D], f32, name=f"yt{qt}", tag="yt")
                nc.vector.tensor_scalar(
                    out=yt, in0=y_sb[:, qt, :], scalar1=mv[:, 0:1],
                    scalar2=mv[:, 1:2], op0=ALU.subtract, op1=ALU.mult,
                )
                nc.vector.tensor_mul(out=yt, in0=yt, in1=lngb)
                nc.vector.tensor_add(out=yt, in0=yt, in1=lnbb)
                nc.sync.dma_start(
                    out=out.ap()[qt * 128:(qt + 1) * 128, :], in_=yt
                )


_NC_CACHE = None


def kernel(**inputs) -> np.ndarray:
    global _NC_CACHE
    x = np.asarray(inputs["x"], dtype=np.float32)
    W_attn = np.asarray(inputs["W_attn"], np.float32)
    b_attn = np.asarray(inputs["b_attn"], np.float32)
    W_out = np.asarray(inputs["W_out"], np.float32)
    b_out = np.asarray(inputs["b_out"], np.float32)
    ln_g = np.asarray(inputs["ln_g"], np.float32)
    ln_b = np.asarray(inputs["ln_b"], np.float32)
    mask = np.asarray(inputs["mask"])

    if _NC_CACHE is None:
        _NC_CACHE = _build()
    nc = _NC_CACHE

    bqa = np.ascontiguousarray(b_attn[0:D].reshape(8, 128).T)
    bka = np.ascontiguousarray(b_attn[D:2 * D].reshape(8, 128).T)
    bva = np.ascontiguousarray(b_attn[2 * D:3 * D].reshape(1, D))
    wqa = np.ascontiguousarray(W_attn[:, 0:D]).astype(ml_dtypes.bfloat16)
    wka = np.ascontiguousarray(W_attn[:, D:2 * D]).astype(ml_dtypes.bfloat16)
    wva = np.ascontiguousarray(W_attn[:, 2 * D:3 * D]).astype(
        ml_dtypes.bfloat16)
    wo = np.ascontiguousarray(W_out).astype(ml_dtypes.bfloat16)
    xTb = [np.ascontiguousarray(x[b].T).astype(ml_dtypes.bfloat16)
           for b in range(B)]

    in_maps = []
    for c in range(NCORES):
        b, r = divmod(c, R)
        rows = slice(SL * r, SL * (r + 1))
        xTql = np.ascontiguousarray(xTb[b][:, rows])
        mKeep = np.ascontiguousarray(
            (~mask[b, 0, rows, :]).T.astype(np.float32)
        ).astype(ml_dtypes.bfloat16)
        xresl = np.ascontiguousarray(x[b, rows, :] + b_out[None, :])
        in_maps.append(dict(
            xT=xTb[b], xTq=xTql, wq=wqa, wk=wka, wv=wva, bq=bqa, bk=bka,
            bv=bva, wout=wo, maskm=mKeep, xres=xresl,
            lng=ln_g.reshape(1, D), lnb=ln_b.reshape(1, D),
        ))

    res = bass_utils.run_bass_kernel_spmd(nc, in_maps,
                                          core_ids=list(range(NCORES)))
    kernel.last_results = res

    full = np.empty((B, S, D), np.float32)
    for c in range(NCORES):
        b, r = divmod(c, R)
        full[b, SL * r:SL * (r + 1), :] = res.results[c]["out"]
    return full


if __name__ == "__main__":
    rng = np.random.default_rng(0)
    ins = dict(
        x=rng.standard_normal((B, S, D), dtype=np.float32),
        W_attn=rng.standard_normal((D, 3 * D), dtype=np.float32) / 32,
        b_attn=np.zeros(3 * D, np.float32),
        W_out=rng.standard_normal((D, D), dtype=np.float32) / 32,
        b_out=np.zeros(D, np.float32),
        ln_g=np.ones(D, np.float32),
        ln_b=np.zeros(D, np.float32),
        mask=rng.integers(0, 5, (B, 1, S, S)) == 0,
    )
    y = kernel(**ins)
    print("ok", y.shape, y.dtype)
